# revision 14
# baseline (speedup 1.0000x reference)
"""Trainium2 Bass kernel for nn_AttentionStem (5x5 local attention stem, stride 2).

Self-contained: hardcodes shapes B=8, CIN=64, H=W=128, OUT_CH=128, M=2, K=5.
Data-parallel over batch: one batch element per NeuronCore (8 cores).

Math (per batch):
  scores[k,(h,w)] = x_s(2h,2w)^T G x(p'_k),  G = w_q^T w_k   (q/k projections folded)
  attn = softmax_k(scores)
  out[c,(h,w)] = sum_k attn_k sum_m wpos[m,k] v[2c+m, p'_k],  v = w_v x

Kernel pipeline per core:
  y = G^T x_s                                   (TensorE, f32r)
  V[p'] = w_v x  in column-partition layout      (TensorE, f32r -> bf16)
  per input row r: ST_r[p', pos] = x_r^T y       (TensorE, transposed dense scores)
  E2T = exp(ST)                                  (ScalarE, PSUM->SBUF bf16)
  A_m = E2T * wpos_mask_m                        (VectorE, banded masking)
  out[pos,c] = sum_{r,m} A_m^T V_m  (+ ones-col -> softmax denominator)  (TensorE)
  scale by 1/denom, PE-transpose to [c,pos], DMA out.
"""

import sys

for _p in ("/opt/pypackages", "/opt/trn_rl_repo"):
    if _p not in sys.path:
        sys.path.insert(0, _p)

from contextlib import ExitStack

import ml_dtypes
import numpy as np

import concourse.bacc as bacc
import concourse.bass as bass
import concourse.mybir as mybir
from concourse.bass_utils import run_bass_kernel_spmd
from concourse.tile import TileContext

F32 = mybir.dt.float32
F32R = mybir.dt.float32r
BF16 = mybir.dt.bfloat16

NCORES = 8
CIN = 64
IMG = 128          # input H = W
PIT = IMG + 4      # padded row pitch (pad 2 each side)
OC = 128           # out channels
VCH = 258          # V row pitch: 256 v-channels + 2 ones columns
HO = 64            # output H = W
NPAIR = 32         # output row pairs

# (t, q) -> d  (d = r - 4j for input row r = 4j + d serving pair j)
D_OF = {(0, 0): 4, (0, 1): 0, (1, 0): 1, (2, 0): 2, (2, 1): -2, (3, 0): 3, (3, 1): -1}
# d -> (t, q)
TQ_OF = {d: tq for tq, d in D_OF.items()}


def make_wpos(row_emb, col_emb, mix_emb):
    a = mix_emb.T.astype(np.float64) @ row_emb.astype(np.float64)  # [2,5]
    b = mix_emb.T.astype(np.float64) @ col_emb.astype(np.float64)  # [2,5]
    wp = a[:, :, None] + b[:, None, :]                             # [2,5,5]
    wp = wp - wp.max(axis=0, keepdims=True)
    e = np.exp(wp)
    wp = e / e.sum(axis=0, keepdims=True)
    return wp.reshape(2, 25).astype(np.float32)                    # [m, dh*5+dw]


def make_masks(wpos):
    """wpos-weighted band masks in the transposed (ST) layout.

    Returns [128 (p'=image col), 2 (m), 1024 (t*256 + q*128 + rho*64 + w)] f32."""
    wm = np.zeros((128, 2, 4, 2, 2, 64), np.float32)
    for (t, q), d in D_OF.items():
        for rho in (0, 1):
            dh = d + 2 - 2 * rho
            if not 0 <= dh < 5:
                continue
            for w in range(64):
                for dw in range(5):
                    cimg = 2 * w + dw - 2
                    if 0 <= cimg < 128:
                        wm[cimg, :, t, q, rho, w] = wpos[:, dh * 5 + dw]
    return wm.reshape(128, 2, 1024)


def make_oob():
    """#window entries with out-of-image column, per position in a pair: exp(0)=1 each."""
    oob = np.zeros((128, 1), np.float32)
    for rho in (0, 1):
        for w in range(64):
            cnt = sum(1 for dw in range(5) if not 0 <= 2 * w + dw - 2 < 128)
            oob[rho * 64 + w, 0] = 5.0 * cnt
    return oob


def _ap(t, off, dims, p0=0, pn=None):
    a = t[:]
    np_ = pn if pn is not None else a.ap[0][1]
    return bass.AP(tensor=a.tensor, offset=off + p0 * a.ap[0][0],
                   ap=[[a.ap[0][0], np_]] + [list(d) for d in dims])


def _aph(t, off, dims, p0, pn):
    a = t[p0:p0 + pn]
    return bass.AP(tensor=a.tensor, offset=a.offset + off, ap=[list(a.ap[0])] + [list(d) for d in dims])


def build_nc():
    nc = bacc.Bacc("TRN2", target_bir_lowering=False, debug=False, num_devices=NCORES)

    x_d = nc.dram_tensor("x", [CIN, IMG, IMG], F32, kind="ExternalInput")
    g_d = nc.dram_tensor("g", [128, CIN], BF16, kind="ExternalInput")
    wvt_d = nc.dram_tensor("wvt", [128, 256], BF16, kind="ExternalInput")
    wm_d = nc.dram_tensor("wmask", [128, 2048], BF16, kind="ExternalInput")
    oob_d = nc.dram_tensor("oob", [128, 1], F32, kind="ExternalInput")
    out_d = nc.dram_tensor("out", [HO * HO, OC], F32, kind="ExternalOutput")

    EXP = mybir.ActivationFunctionType.Exp

    with TileContext(nc) as tc, ExitStack() as ctx:
        sg = ctx.enter_context(tc.tile_pool(name="singles", bufs=1))
        x_pad = sg.tile([128, PIT * PIT], BF16)      # padded image, f32
        v_sb = sg.tile([128, PIT * VCH], BF16)       # V + ones cols, padded rows
        y_sb = sg.tile([128, 4096], BF16)            # queries at strided positions
        wm_sb = sg.tile([128, 2048], BF16)
        oob_sb = sg.tile([128, 1], F32)
        g_sb = sg.tile([128, CIN], BF16)
        wvt_sb = sg.tile([128, 256], BF16)

        # constant loads
        nc.sync.dma_start(out=g_sb[:], in_=g_d.ap())
        nc.sync.dma_start(out=wvt_sb[:], in_=wvt_d.ap())
        nc.sync.dma_start(out=wm_sb[:], in_=wm_d.ap())
        nc.sync.dma_start(out=oob_sb[:], in_=oob_d.ap())

        # x_pad borders: rows 0,1 and 130,131; cols 0,1 and 130,131 of interior rows
        nc.vector.memset(_ap(x_pad, 0, [[1, 2 * PIT]]), 0.0)
        nc.vector.memset(_ap(x_pad, 130 * PIT, [[1, 2 * PIT]]), 0.0)
        nc.vector.memset(_ap(x_pad, 2 * PIT, [[PIT, 128], [1, 2]]), 0.0)
        nc.vector.memset(_ap(x_pad, 2 * PIT + 130, [[PIT, 128], [1, 2]]), 0.0)

        # V pad rows (zero) + ones columns
        nc.vector.memset(_ap(v_sb, 0, [[1, 2 * VCH]]), 0.0)
        nc.vector.memset(_ap(v_sb, 130 * VCH, [[1, 2 * VCH]]), 0.0)
        nc.vector.memset(_ap(v_sb, 256, [[VCH, PIT], [1, 2]]), 1.0)

        # x load: 4 chunks of 32 rows, duplicated on both partition halves
        for c4 in range(4):
            nc.gpsimd.dma_start(
                out=_aph(x_pad, (2 + 32 * c4) * PIT + 2, [[PIT, 32], [1, IMG]], 0, 64),
                in_=x_d.ap()[:, 32 * c4:32 * c4 + 32, :],
            )

        def xrow(r, h):
            return x_pad[h:h + 64, (r + 2) * PIT + 2:(r + 2) * PIT + 2 + IMG]

        # ---- phase y + V (shared PSUM pool) ----
        with tc.tile_pool(name="vps", bufs=2, space="PSUM") as vps:
            for ch in range(2):
                yp = vps.tile([128, 2048], F32, tag="vp")
                for i in range(4):
                    hs0 = ch * 32 + i * 8
                    h = 0 * (i % 2)
                    rhs = _aph(x_pad, (2 * hs0 + 2) * PIT + 2, [[2 * PIT, 8], [2, 64]], h, 64)
                    nc.tensor.matmul(yp[0:CIN, i * 512:(i + 1) * 512], g_sb[h:h + 64, :], rhs,
                                     start=True, stop=True)
                nc.vector.tensor_copy(y_sb[0:CIN, ch * 2048:(ch + 1) * 2048], yp[0:CIN, :])


            for vs in range(16):
                vp = vps.tile([128, 2048], F32)
                for i in range(8):
                    r = vs * 8 + i
                    nc.tensor.matmul(vp[:, i * 256:(i + 1) * 256], xrow(r, 0),
                                     wvt_sb[0:64, :], start=True, stop=True)
                dst = _ap(v_sb, (vs * 8 + 2) * VCH, [[VCH, 8], [1, 256]])
                src = vp[:].rearrange("p (r c) -> p r c", c=256)
                if vs % 3 == 0:
                    nc.vector.tensor_copy(dst, src)
                else:
                    nc.scalar.copy(dst, src)

        # ---- phase pairs: transposed scores slabs + apply ----
        with tc.tile_pool(name="stps", bufs=2, space="PSUM") as stps, \
             tc.tile_pool(name="aps", bufs=3, space="PSUM") as aps, \
             tc.tile_pool(name="e2t", bufs=3) as e2t, \
             tc.tile_pool(name="a0p", bufs=4) as a0p, \
             tc.tile_pool(name="a1p", bufs=4) as a1p, \
             tc.tile_pool(name="outsb", bufs=4) as outsb, \
             tc.tile_pool(name="dens", bufs=6) as dens:

            A = {}

            def make_slab(s):
                stp = stps.tile([128, 1024], F32)
                for t in range(4):
                    r = 4 * s + t
                    if t == 0:
                        jmin, col0 = s - 1, 0
                        if s == 0:
                            jmin, col0 = 0, 128
                    else:
                        jmin, col0 = s, 0
                    n = min(256 - col0, (NPAIR - jmin) * 128)
                    h = 0
                    rhs = y_sb[h:h + 64, jmin * 128: jmin * 128 + n]
                    nc.tensor.matmul(stp[:, t * 256 + col0: t * 256 + col0 + n],
                                     xrow(r, h), rhs, start=True, stop=True)
                e2 = e2t.tile([128, 1024], BF16)
                nc.scalar.activation(out=e2[:, 0:384], in_=stp[:, 0:384], func=EXP)
                nc.scalar.activation(out=e2[:, 512:1024], in_=stp[:, 512:1024], func=EXP)
                a0 = a0p.tile([128, 1024], BF16)
                a1 = a1p.tile([128, 1024], BF16)
                nc.vector.tensor_mul(a0[:], e2[:], wm_sb[:, 0:1024])
                nc.vector.tensor_mul(a1[:], e2[:], wm_sb[:, 1024:2048])
                A[s] = (a0, a1)
                A.pop(s - 3, None)

            def apply_pair(j):
                ap_ps = aps.tile([128, 130], F32)
                HALF_RHO = {-2: 0, -1: 0, 3: 1, 4: 1}
                ops = [(d, m) for d in (0, 1, 2, -2, -1, 3, 4) for m in (0, 1)]
                for idx, (d, m) in enumerate(ops):
                    r = 4 * j + d
                    if r < 0 or r >= IMG:
                        t, q = TQ_OF[d]
                        off = m * 1024 + t * 256 + q * 128
                        src = wm_sb
                    else:
                        sl, t = r // 4, r % 4
                        if t == 0:
                            q = 0 if j == sl - 1 else 1
                        elif t == 1:
                            q = 0
                        else:
                            q = 0 if j == sl else 1
                        off = t * 256 + q * 128
                        src = A[sl][m]
                    rho = HALF_RHO.get(d)
                    if rho is None:
                        lhsT = src[:, off: off + 128]
                        out_ps = ap_ps[:, 0:129]
                    else:
                        lhsT = src[:, off + rho * 64: off + rho * 64 + 64]
                        out_ps = ap_ps[rho * 64: rho * 64 + 64, 0:129]
                    rhs = _ap(v_sb, (r + 2) * VCH + m, [[2, 129]])
                    nc.tensor.matmul(out_ps, lhsT, rhs,
                                     start=(idx == 0), stop=(idx == len(ops) - 1),
                                     skip_group_check=True)
                den = dens.tile([128, 1], F32)
                nc.vector.tensor_add(den[:], ap_ps[:, 128:129], oob_sb[:])
                rec = dens.tile([128, 1], F32)
                nc.vector.reciprocal(rec[:], den[:])
                o_sb = outsb.tile([128, 128], F32)
                nc.vector.tensor_scalar_mul(o_sb[:], ap_ps[:, 0:128], rec[:])
                nc.sync.dma_start(out=out_d.ap()[j * 128:(j + 1) * 128, :], in_=o_sb[:])

            for s in range(NPAIR):
                make_slab(s)
                if s >= 1:
                    apply_pair(s - 1)
            apply_pair(NPAIR - 1)


    nc.compile()
    return nc


_NC_CACHE = None


def kernel(x, w_q, w_k, w_v, row_emb, col_emb, mix_emb):
    global _NC_CACHE
    x = np.asarray(x, np.float32)
    w_q = np.asarray(w_q, np.float32)
    w_k = np.asarray(w_k, np.float32)
    w_v = np.asarray(w_v, np.float32)
    row_emb = np.asarray(row_emb, np.float32)
    col_emb = np.asarray(col_emb, np.float32)
    mix_emb = np.asarray(mix_emb, np.float32)

    G = (w_q.T @ w_k).astype(np.float32)
    wvt = np.ascontiguousarray(w_v.T).astype(np.float32)      # [64, 256]
    wpos = make_wpos(row_emb, col_emb, mix_emb)
    wmask = make_masks(wpos).reshape(128, 2048).astype(ml_dtypes.bfloat16)
    oob = make_oob()
    ident = np.eye(128, dtype=np.float32)

    if _NC_CACHE is None:
        _NC_CACHE = build_nc()
    nc = _NC_CACHE

    in_maps = []
    for b in range(NCORES):
        in_maps.append({
            "x": np.ascontiguousarray(x[b]),
            "g": G,
            "wvt": wvt,
            "wmask": wmask,
            "oob": oob,
        })
    res = run_bass_kernel_spmd(nc, in_maps, core_ids=list(range(NCORES)))
    out = np.stack([res.results[b]["out"].T.reshape(OC, HO, HO) for b in range(NCORES)])
    return out.astype(np.float32)


# revision 15
# speedup vs baseline: 1.4531x; 1.4531x over previous
"""Trainium2 Bass kernel for nn_AttentionStem (5x5 local attention stem, stride 2).

Self-contained: hardcodes shapes B=8, CIN=64, H=W=128, OUT_CH=128, M=2, K=5.
Data-parallel over batch: one batch element per NeuronCore (8 cores).

Math (per batch):
  scores[k,(h,w)] = x_s(2h,2w)^T G x(p'_k),  G = w_q^T w_k   (q/k projections folded)
  attn = softmax_k(scores)
  out[c,(h,w)] = sum_k attn_k sum_m wpos[m,k] v[2c+m, p'_k],  v = w_v x

Kernel pipeline per core:
  y = G^T x_s                                   (TensorE, f32r)
  V[p'] = w_v x  in column-partition layout      (TensorE, f32r -> bf16)
  per input row r: ST_r[p', pos] = x_r^T y       (TensorE, transposed dense scores)
  E2T = exp(ST)                                  (ScalarE, PSUM->SBUF bf16)
  A_m = E2T * wpos_mask_m                        (VectorE, banded masking)
  out[pos,c] = sum_{r,m} A_m^T V_m  (+ ones-col -> softmax denominator)  (TensorE)
  scale by 1/denom, PE-transpose to [c,pos], DMA out.
"""

import sys

for _p in ("/opt/pypackages", "/opt/trn_rl_repo"):
    if _p not in sys.path:
        sys.path.insert(0, _p)

from contextlib import ExitStack

import ml_dtypes
import numpy as np

import concourse.bacc as bacc
import concourse.bass as bass
import concourse.mybir as mybir
from concourse.bass_utils import run_bass_kernel_spmd
from concourse.tile import TileContext

F32 = mybir.dt.float32
F32R = mybir.dt.float32r
BF16 = mybir.dt.bfloat16

NCORES = 8
CIN = 64
IMG = 128          # input H = W
PIT = IMG + 4      # padded row pitch (pad 2 each side)
OC = 128           # out channels
VCH = 258          # V row pitch: 256 v-channels + 2 ones columns
HO = 64            # output H = W
NPAIR = 32         # output row pairs

# (t, q) -> d  (d = r - 4j for input row r = 4j + d serving pair j)
D_OF = {(0, 0): 4, (0, 1): 0, (1, 0): 1, (2, 0): 2, (2, 1): -2, (3, 0): 3, (3, 1): -1}
# d -> (t, q)
TQ_OF = {d: tq for tq, d in D_OF.items()}


def make_wpos(row_emb, col_emb, mix_emb):
    a = mix_emb.T.astype(np.float64) @ row_emb.astype(np.float64)  # [2,5]
    b = mix_emb.T.astype(np.float64) @ col_emb.astype(np.float64)  # [2,5]
    wp = a[:, :, None] + b[:, None, :]                             # [2,5,5]
    wp = wp - wp.max(axis=0, keepdims=True)
    e = np.exp(wp)
    wp = e / e.sum(axis=0, keepdims=True)
    return wp.reshape(2, 25).astype(np.float32)                    # [m, dh*5+dw]


def make_masks(wpos):
    """wpos-weighted band masks in the transposed (ST) layout.

    Returns [128 (p'=image col), 2 (m), 1024 (t*256 + q*128 + rho*64 + w)] f32."""
    wm = np.zeros((128, 2, 4, 2, 2, 64), np.float32)
    for (t, q), d in D_OF.items():
        for rho in (0, 1):
            dh = d + 2 - 2 * rho
            if not 0 <= dh < 5:
                continue
            for w in range(64):
                for dw in range(5):
                    cimg = 2 * w + dw - 2
                    if 0 <= cimg < 128:
                        wm[cimg, :, t, q, rho, w] = wpos[:, dh * 5 + dw]
    return wm.reshape(128, 2, 1024)


def make_oob():
    """#window entries with out-of-image column, per position in a pair: exp(0)=1 each."""
    oob = np.zeros((128, 1), np.float32)
    for rho in (0, 1):
        for w in range(64):
            cnt = sum(1 for dw in range(5) if not 0 <= 2 * w + dw - 2 < 128)
            oob[rho * 64 + w, 0] = 5.0 * cnt
    return oob


def _ap(t, off, dims, p0=0, pn=None):
    a = t[:]
    np_ = pn if pn is not None else a.ap[0][1]
    return bass.AP(tensor=a.tensor, offset=off + p0 * a.ap[0][0],
                   ap=[[a.ap[0][0], np_]] + [list(d) for d in dims])


def _aph(t, off, dims, p0, pn):
    a = t[p0:p0 + pn]
    return bass.AP(tensor=a.tensor, offset=a.offset + off, ap=[list(a.ap[0])] + [list(d) for d in dims])


def build_nc():
    nc = bacc.Bacc("TRN2", target_bir_lowering=False, debug=False, num_devices=NCORES)

    x_d = nc.dram_tensor("x", [CIN, IMG, IMG], F32, kind="ExternalInput")
    g_d = nc.dram_tensor("g", [128, CIN], BF16, kind="ExternalInput")
    wvt_d = nc.dram_tensor("wvt", [128, 256], BF16, kind="ExternalInput")
    wm_d = nc.dram_tensor("wmask", [128, 2048], BF16, kind="ExternalInput")
    oob_d = nc.dram_tensor("oob", [128, 1], F32, kind="ExternalInput")
    out_d = nc.dram_tensor("out", [HO * HO, OC], F32, kind="ExternalOutput")

    EXP = mybir.ActivationFunctionType.Exp

    with TileContext(nc) as tc, ExitStack() as ctx:
        sg = ctx.enter_context(tc.tile_pool(name="singles", bufs=1))
        x_pad = sg.tile([128, PIT * PIT], BF16)      # padded image, f32
        v_sb = sg.tile([128, PIT * VCH], BF16)       # V + ones cols, padded rows
        y_sb = sg.tile([128, 4096], BF16)            # queries at strided positions
        wm_sb = sg.tile([128, 2048], BF16)
        oob_sb = sg.tile([128, 1], F32)
        g_sb = sg.tile([128, CIN], BF16)
        wvt_sb = sg.tile([128, 256], BF16)

        # constant loads
        nc.sync.dma_start(out=g_sb[:], in_=g_d.ap())
        nc.sync.dma_start(out=wvt_sb[:], in_=wvt_d.ap())
        nc.sync.dma_start(out=wm_sb[:], in_=wm_d.ap())
        nc.sync.dma_start(out=oob_sb[:], in_=oob_d.ap())

        # x_pad borders: rows 0,1 and 130,131; cols 0,1 and 130,131 of interior rows
        nc.vector.memset(_ap(x_pad, 0, [[1, 2 * PIT]]), 0.0)
        nc.vector.memset(_ap(x_pad, 130 * PIT, [[1, 2 * PIT]]), 0.0)
        nc.vector.memset(_ap(x_pad, 2 * PIT, [[PIT, 128], [1, 2]]), 0.0)
        nc.vector.memset(_ap(x_pad, 2 * PIT + 130, [[PIT, 128], [1, 2]]), 0.0)

        # V pad rows (zero) + ones columns
        nc.vector.memset(_ap(v_sb, 0, [[1, 2 * VCH]]), 0.0)
        nc.vector.memset(_ap(v_sb, 130 * VCH, [[1, 2 * VCH]]), 0.0)
        nc.vector.memset(_ap(v_sb, 256, [[VCH, PIT], [1, 2]]), 1.0)

        # x load: 4 chunks of 32 rows, duplicated on both partition halves
        for c4 in range(4):
            nc.gpsimd.dma_start(
                out=_aph(x_pad, (2 + 32 * c4) * PIT + 2, [[PIT, 32], [1, IMG]], 0, 64),
                in_=x_d.ap()[:, 32 * c4:32 * c4 + 32, :],
            )

        def xrow(r, h):
            return x_pad[h:h + 64, (r + 2) * PIT + 2:(r + 2) * PIT + 2 + IMG]

        # ---- phase y + V (shared PSUM pool) ----
        with tc.tile_pool(name="vps", bufs=2, space="PSUM") as vps:
            for ch in range(2):
                yp = vps.tile([128, 2048], F32, tag="vp")
                for i in range(4):
                    hs0 = ch * 32 + i * 8
                    h = 0 * (i % 2)
                    rhs = _aph(x_pad, (2 * hs0 + 2) * PIT + 2, [[2 * PIT, 8], [2, 64]], h, 64)
                    nc.tensor.matmul(yp[0:CIN, i * 512:(i + 1) * 512], g_sb[h:h + 64, :], rhs,
                                     start=True, stop=True)
                nc.vector.tensor_copy(y_sb[0:CIN, ch * 2048:(ch + 1) * 2048], yp[0:CIN, :])


            for vs in range(16):
                vp = vps.tile([128, 2048], F32)
                for i in range(8):
                    r = vs * 8 + i
                    nc.tensor.matmul(vp[:, i * 256:(i + 1) * 256], xrow(r, 0),
                                     wvt_sb[0:64, :], start=True, stop=True)
                dst = _ap(v_sb, (vs * 8 + 2) * VCH, [[VCH, 8], [1, 256]])
                src = vp[:].rearrange("p (r c) -> p r c", c=256)
                if vs % 3 == 0:
                    nc.vector.tensor_copy(dst, src)
                else:
                    nc.scalar.copy(dst, src)

        # ---- phase pairs: transposed scores slabs + apply ----
        with tc.tile_pool(name="stps", bufs=2, space="PSUM") as stps, \
             tc.tile_pool(name="aps", bufs=3, space="PSUM") as aps, \
             tc.tile_pool(name="e2t", bufs=3) as e2t, \
             tc.tile_pool(name="a0p", bufs=4) as a0p, \
             tc.tile_pool(name="a1p", bufs=4) as a1p, \
             tc.tile_pool(name="outsb", bufs=4) as outsb, \
             tc.tile_pool(name="dens", bufs=6) as dens:

            A = {}

            def make_slab(s):
                stp = stps.tile([128, 1024], F32)
                for t in range(4):
                    r = 4 * s + t
                    if t == 0:
                        jmin, col0 = s - 1, 0
                        if s == 0:
                            jmin, col0 = 0, 128
                    else:
                        jmin, col0 = s, 0
                    n = min(256 - col0, (NPAIR - jmin) * 128)
                    h = 0
                    rhs = y_sb[h:h + 64, jmin * 128: jmin * 128 + n]
                    nc.tensor.matmul(stp[:, t * 256 + col0: t * 256 + col0 + n],
                                     xrow(r, h), rhs, start=True, stop=True)
                e2 = e2t.tile([128, 1024], BF16)
                nc.scalar.activation(out=e2[:], in_=stp[:], func=EXP)
                a0 = a0p.tile([128, 1024], BF16)
                a1 = a1p.tile([128, 1024], BF16)
                nc.vector.tensor_mul(a0[:], e2[:], wm_sb[:, 0:1024])
                nc.vector.tensor_mul(a1[:], e2[:], wm_sb[:, 1024:2048])
                A[s] = (a0, a1)
                A.pop(s - 3, None)

            def apply_pair(j):
                ap_ps = aps.tile([128, 130], F32)
                HALF_RHO = {-2: 0, -1: 0, 3: 1, 4: 1}
                ops = [(d, m) for d in (0, 1, 2, -2, -1, 3, 4) for m in (0, 1)]
                for idx, (d, m) in enumerate(ops):
                    r = 4 * j + d
                    if r < 0 or r >= IMG:
                        t, q = TQ_OF[d]
                        off = m * 1024 + t * 256 + q * 128
                        src = wm_sb
                    else:
                        sl, t = r // 4, r % 4
                        if t == 0:
                            q = 0 if j == sl - 1 else 1
                        elif t == 1:
                            q = 0
                        else:
                            q = 0 if j == sl else 1
                        off = t * 256 + q * 128
                        src = A[sl][m]
                    lhsT = src[:, off: off + 128]
                    out_ps = ap_ps[:, 0:129]
                    rhs = _ap(v_sb, (r + 2) * VCH + m, [[2, 129]])
                    nc.tensor.matmul(out_ps, lhsT, rhs,
                                     start=(idx == 0), stop=(idx == len(ops) - 1),
                                     skip_group_check=True)
                den = dens.tile([128, 1], F32)
                nc.vector.tensor_add(den[:], ap_ps[:, 128:129], oob_sb[:])
                rec = dens.tile([128, 1], F32)
                nc.vector.reciprocal(rec[:], den[:])
                o_sb = outsb.tile([128, 128], F32)
                nc.vector.tensor_scalar_mul(o_sb[:], ap_ps[:, 0:128], rec[:])
                nc.sync.dma_start(out=out_d.ap()[j * 128:(j + 1) * 128, :], in_=o_sb[:])

            for s in range(NPAIR):
                make_slab(s)
                if s >= 1:
                    apply_pair(s - 1)
            apply_pair(NPAIR - 1)


    nc.compile()
    return nc


_NC_CACHE = None


def kernel(x, w_q, w_k, w_v, row_emb, col_emb, mix_emb):
    global _NC_CACHE
    x = np.asarray(x, np.float32)
    w_q = np.asarray(w_q, np.float32)
    w_k = np.asarray(w_k, np.float32)
    w_v = np.asarray(w_v, np.float32)
    row_emb = np.asarray(row_emb, np.float32)
    col_emb = np.asarray(col_emb, np.float32)
    mix_emb = np.asarray(mix_emb, np.float32)

    G = (w_q.T @ w_k).astype(np.float32)
    wvt = np.ascontiguousarray(w_v.T).astype(np.float32)      # [64, 256]
    wpos = make_wpos(row_emb, col_emb, mix_emb)
    wmask = make_masks(wpos).reshape(128, 2048).astype(ml_dtypes.bfloat16)
    oob = make_oob()
    ident = np.eye(128, dtype=np.float32)

    if _NC_CACHE is None:
        _NC_CACHE = build_nc()
    nc = _NC_CACHE

    in_maps = []
    for b in range(NCORES):
        in_maps.append({
            "x": np.ascontiguousarray(x[b]),
            "g": G,
            "wvt": wvt,
            "wmask": wmask,
            "oob": oob,
        })
    res = run_bass_kernel_spmd(nc, in_maps, core_ids=list(range(NCORES)))
    out = np.stack([res.results[b]["out"].T.reshape(OC, HO, HO) for b in range(NCORES)])
    return out.astype(np.float32)


# revision 18
# speedup vs baseline: 1.4839x; 1.0212x over previous
"""Trainium2 Bass kernel for nn_AttentionStem (5x5 local attention stem, stride 2).

Self-contained: hardcodes shapes B=8, CIN=64, H=W=128, OUT_CH=128, M=2, K=5.
Data-parallel over batch: one batch element per NeuronCore (8 cores).

Math (per batch):
  scores[k,(h,w)] = x_s(2h,2w)^T G x(p'_k),  G = w_q^T w_k   (q/k projections folded)
  attn = softmax_k(scores)
  out[c,(h,w)] = sum_k attn_k sum_m wpos[m,k] v[2c+m, p'_k],  v = w_v x

Kernel pipeline per core:
  y = G^T x_s                                   (TensorE, f32r)
  V[p'] = w_v x  in column-partition layout      (TensorE, f32r -> bf16)
  per input row r: ST_r[p', pos] = x_r^T y       (TensorE, transposed dense scores)
  E2T = exp(ST)                                  (ScalarE, PSUM->SBUF bf16)
  A_m = E2T * wpos_mask_m                        (VectorE, banded masking)
  out[pos,c] = sum_{r,m} A_m^T V_m  (+ ones-col -> softmax denominator)  (TensorE)
  scale by 1/denom, PE-transpose to [c,pos], DMA out.
"""

import sys

for _p in ("/opt/pypackages", "/opt/trn_rl_repo"):
    if _p not in sys.path:
        sys.path.insert(0, _p)

from contextlib import ExitStack

import ml_dtypes
import numpy as np

import concourse.bacc as bacc
import concourse.bass as bass
import concourse.mybir as mybir
from concourse.bass_utils import run_bass_kernel_spmd
from concourse.tile import TileContext

F32 = mybir.dt.float32
BF16 = mybir.dt.bfloat16

NCORES = 8
CIN = 64
IMG = 128          # input H = W
PIT = IMG + 4      # padded row pitch (pad 2 each side)
OC = 128           # out channels
VCH = 258          # V row pitch: 256 v-channels + 2 ones columns
HO = 64            # output H = W
NPAIR = 32         # output row pairs

# (t, q) -> d  (d = r - 4j for input row r = 4j + d serving pair j)
D_OF = {(0, 0): 4, (0, 1): 0, (1, 0): 1, (2, 0): 2, (2, 1): -2, (3, 0): 3, (3, 1): -1}
# d -> (t, q)
TQ_OF = {d: tq for tq, d in D_OF.items()}


def make_wpos(row_emb, col_emb, mix_emb):
    a = mix_emb.T.astype(np.float64) @ row_emb.astype(np.float64)  # [2,5]
    b = mix_emb.T.astype(np.float64) @ col_emb.astype(np.float64)  # [2,5]
    wp = a[:, :, None] + b[:, None, :]                             # [2,5,5]
    wp = wp - wp.max(axis=0, keepdims=True)
    e = np.exp(wp)
    wp = e / e.sum(axis=0, keepdims=True)
    return wp.reshape(2, 25).astype(np.float32)                    # [m, dh*5+dw]


def make_masks(wpos):
    """wpos-weighted band masks in the transposed (ST) layout.

    Returns [128 (p'=image col), 2 (m), 1024 (t*256 + q*128 + rho*64 + w)] f32."""
    wm = np.zeros((128, 2, 4, 2, 2, 64), np.float32)
    for (t, q), d in D_OF.items():
        for rho in (0, 1):
            dh = d + 2 - 2 * rho
            if not 0 <= dh < 5:
                continue
            for w in range(64):
                for dw in range(5):
                    cimg = 2 * w + dw - 2
                    if 0 <= cimg < 128:
                        wm[cimg, :, t, q, rho, w] = wpos[:, dh * 5 + dw]
    return wm.reshape(128, 2, 1024)


def make_oob():
    """#window entries with out-of-image column, per position in a pair: exp(0)=1 each."""
    oob = np.zeros((128, 1), np.float32)
    for rho in (0, 1):
        for w in range(64):
            cnt = sum(1 for dw in range(5) if not 0 <= 2 * w + dw - 2 < 128)
            oob[rho * 64 + w, 0] = 5.0 * cnt
    return oob


def _ap(t, off, dims, p0=0, pn=None):
    a = t[:]
    np_ = pn if pn is not None else a.ap[0][1]
    return bass.AP(tensor=a.tensor, offset=off + p0 * a.ap[0][0],
                   ap=[[a.ap[0][0], np_]] + [list(d) for d in dims])


def _aph(t, off, dims, p0, pn):
    a = t[p0:p0 + pn]
    return bass.AP(tensor=a.tensor, offset=a.offset + off, ap=[list(a.ap[0])] + [list(d) for d in dims])


def build_nc():
    nc = bacc.Bacc("TRN2", target_bir_lowering=False, debug=False, num_devices=NCORES)

    x_d = nc.dram_tensor("x", [CIN, IMG, IMG], F32, kind="ExternalInput")
    g_d = nc.dram_tensor("g", [128, CIN], BF16, kind="ExternalInput")
    wvt_d = nc.dram_tensor("wvt", [128, 256], BF16, kind="ExternalInput")
    wm_d = nc.dram_tensor("wmask", [128, 2048], BF16, kind="ExternalInput")
    oob_d = nc.dram_tensor("oob", [128, 1], F32, kind="ExternalInput")
    out_d = nc.dram_tensor("out", [HO * HO, OC], F32, kind="ExternalOutput")

    EXP = mybir.ActivationFunctionType.Exp

    with TileContext(nc) as tc, ExitStack() as ctx:
        sg = ctx.enter_context(tc.tile_pool(name="singles", bufs=1))
        x_pad = sg.tile([128, PIT * PIT], BF16)      # padded image, f32
        v_sb = sg.tile([128, PIT * VCH], BF16)       # V + ones cols, padded rows
        y_sb = sg.tile([128, 4096], BF16)            # queries at strided positions
        wm_sb = sg.tile([128, 2048], BF16)
        oob_sb = sg.tile([128, 1], F32)
        g_sb = sg.tile([128, CIN], BF16)
        wvt_sb = sg.tile([128, 256], BF16)

        # constant loads
        nc.sync.dma_start(out=g_sb[:], in_=g_d.ap())
        nc.sync.dma_start(out=wvt_sb[:], in_=wvt_d.ap())
        nc.sync.dma_start(out=wm_sb[:], in_=wm_d.ap())
        nc.sync.dma_start(out=oob_sb[:], in_=oob_d.ap())

        # x_pad borders: rows 0,1 and 130,131; cols 0,1 and 130,131 of interior rows
        nc.vector.memset(_ap(x_pad, 0, [[1, 2 * PIT]]), 0.0)
        nc.vector.memset(_ap(x_pad, 130 * PIT, [[1, 2 * PIT]]), 0.0)
        nc.vector.memset(_ap(x_pad, 2 * PIT, [[PIT, 128], [1, 2]]), 0.0)
        nc.vector.memset(_ap(x_pad, 2 * PIT + 130, [[PIT, 128], [1, 2]]), 0.0)

        # V pad rows (zero) + ones columns
        nc.vector.memset(_ap(v_sb, 0, [[1, 2 * VCH]]), 0.0)
        nc.vector.memset(_ap(v_sb, 130 * VCH, [[1, 2 * VCH]]), 0.0)
        nc.vector.memset(_ap(v_sb, 256, [[VCH, PIT], [1, 2]]), 1.0)

        # x load: 4 chunks of 32 rows, duplicated on both partition halves
        for c4 in range(4):
            nc.gpsimd.dma_start(
                out=_aph(x_pad, (2 + 32 * c4) * PIT + 2, [[PIT, 32], [1, IMG]], 0, 64),
                in_=x_d.ap()[:, 32 * c4:32 * c4 + 32, :],
            )

        def xrow(r, h):
            return x_pad[h:h + 64, (r + 2) * PIT + 2:(r + 2) * PIT + 2 + IMG]

        # ---- phase y + V (shared PSUM pool) ----
        with tc.tile_pool(name="vps", bufs=2, space="PSUM") as vps:
            for ch in range(4):
                yp = vps.tile([128, 1024], F32, tag="vph")
                for i in range(2):
                    hs0 = ch * 16 + i * 8
                    rhs = _aph(x_pad, (2 * hs0 + 2) * PIT + 2, [[2 * PIT, 8], [2, 64]], 0, 64)
                    nc.tensor.matmul(yp[0:CIN, i * 512:(i + 1) * 512], g_sb[0:64, :], rhs,
                                     start=True, stop=True)
                nc.vector.tensor_copy(y_sb[0:CIN, ch * 1024:(ch + 1) * 1024], yp[0:CIN, :])


            for vs in range(32):
                vp = vps.tile([128, 1024], F32, tag="vph")
                for i in range(4):
                    r = vs * 4 + i
                    nc.tensor.matmul(vp[:, i * 256:(i + 1) * 256], xrow(r, 0),
                                     wvt_sb[0:64, :], start=True, stop=True)
                dst = _ap(v_sb, (vs * 4 + 2) * VCH, [[VCH, 4], [1, 256]])
                src = vp[:].rearrange("p (r c) -> p r c", c=256)
                if vs % 3 == 0:
                    nc.vector.tensor_copy(dst, src)
                else:
                    nc.scalar.copy(dst, src)

        # ---- phase pairs: transposed scores slabs + apply ----
        with tc.tile_pool(name="stps", bufs=2, space="PSUM") as stps, \
             tc.tile_pool(name="aps", bufs=3, space="PSUM") as aps, \
             tc.tile_pool(name="e2t", bufs=3) as e2t, \
             tc.tile_pool(name="a0p", bufs=4) as a0p, \
             tc.tile_pool(name="a1p", bufs=4) as a1p, \
             tc.tile_pool(name="outsb", bufs=4) as outsb, \
             tc.tile_pool(name="dens", bufs=6) as dens:

            A = {}

            def make_slab(s):
                stp = stps.tile([128, 1024], F32)
                for t in range(4):
                    r = 4 * s + t
                    if t == 0:
                        jmin, col0 = s - 1, 0
                        if s == 0:
                            jmin, col0 = 0, 128
                    else:
                        jmin, col0 = s, 0
                    n = min(256 - col0, (NPAIR - jmin) * 128)
                    h = 0
                    rhs = y_sb[h:h + 64, jmin * 128: jmin * 128 + n]
                    nc.tensor.matmul(stp[:, t * 256 + col0: t * 256 + col0 + n],
                                     xrow(r, h), rhs, start=True, stop=True)
                e2 = e2t.tile([128, 1024], BF16)
                nc.scalar.activation(out=e2[:], in_=stp[:], func=EXP)
                a0 = a0p.tile([128, 1024], BF16)
                a1 = a1p.tile([128, 1024], BF16)
                nc.vector.tensor_mul(a0[:], e2[:], wm_sb[:, 0:1024])
                nc.vector.tensor_mul(a1[:], e2[:], wm_sb[:, 1024:2048])
                A[s] = (a0, a1)
                A.pop(s - 3, None)

            def apply_pair(j):
                ap_ps = aps.tile([128, 130], F32)
                ops = [(d, m) for d in (0, 1, 2, -2, -1, 3, 4) for m in (0, 1)]
                for idx, (d, m) in enumerate(ops):
                    r = 4 * j + d
                    if r < 0 or r >= IMG:
                        t, q = TQ_OF[d]
                        off = m * 1024 + t * 256 + q * 128
                        src = wm_sb
                    else:
                        sl, t = r // 4, r % 4
                        if t == 0:
                            q = 0 if j == sl - 1 else 1
                        elif t == 1:
                            q = 0
                        else:
                            q = 0 if j == sl else 1
                        off = t * 256 + q * 128
                        src = A[sl][m]
                    lhsT = src[:, off: off + 128]
                    out_ps = ap_ps[:, 0:129]
                    rhs = _ap(v_sb, (r + 2) * VCH + m, [[2, 129]])
                    nc.tensor.matmul(out_ps, lhsT, rhs,
                                     start=(idx == 0), stop=(idx == len(ops) - 1),
                                     skip_group_check=True)
                den = dens.tile([128, 1], F32)
                nc.vector.tensor_add(den[:], ap_ps[:, 128:129], oob_sb[:])
                rec = dens.tile([128, 1], F32)
                nc.vector.reciprocal(rec[:], den[:])
                o_sb = outsb.tile([128, 128], F32)
                nc.scalar.activation(out=o_sb[:], in_=ap_ps[:, 0:128],
                                     func=mybir.ActivationFunctionType.Copy, scale=rec[:])
                nc.sync.dma_start(out=out_d.ap()[j * 128:(j + 1) * 128, :], in_=o_sb[:])

            for s in range(NPAIR):
                make_slab(s)
                if s >= 1:
                    apply_pair(s - 1)
            apply_pair(NPAIR - 1)


    nc.compile()
    return nc


_NC_CACHE = None


def kernel(x, w_q, w_k, w_v, row_emb, col_emb, mix_emb):
    global _NC_CACHE
    x = np.asarray(x, np.float32)
    w_q = np.asarray(w_q, np.float32)
    w_k = np.asarray(w_k, np.float32)
    w_v = np.asarray(w_v, np.float32)
    row_emb = np.asarray(row_emb, np.float32)
    col_emb = np.asarray(col_emb, np.float32)
    mix_emb = np.asarray(mix_emb, np.float32)

    G = (w_q.T @ w_k).astype(np.float32)
    wvt = np.ascontiguousarray(w_v.T).astype(np.float32)      # [64, 256]
    wpos = make_wpos(row_emb, col_emb, mix_emb)
    wmask = make_masks(wpos).reshape(128, 2048).astype(ml_dtypes.bfloat16)
    oob = make_oob()
    ident = np.eye(128, dtype=np.float32)

    if _NC_CACHE is None:
        _NC_CACHE = build_nc()
    nc = _NC_CACHE

    in_maps = []
    for b in range(NCORES):
        in_maps.append({
            "x": np.ascontiguousarray(x[b]),
            "g": G,
            "wvt": wvt,
            "wmask": wmask,
            "oob": oob,
        })
    res = run_bass_kernel_spmd(nc, in_maps, core_ids=list(range(NCORES)))
    out = np.stack([res.results[b]["out"].T.reshape(OC, HO, HO) for b in range(NCORES)])
    return out.astype(np.float32)


# revision 19
# speedup vs baseline: 1.5790x; 1.0641x over previous
"""Trainium2 Bass kernel for nn_AttentionStem (5x5 local attention stem, stride 2).

Self-contained: hardcodes shapes B=8, CIN=64, H=W=128, OUT_CH=128, M=2, K=5.
Data-parallel over batch: one batch element per NeuronCore (8 cores).

Math (per batch):
  scores[k,(h,w)] = x_s(2h,2w)^T G x(p'_k),  G = w_q^T w_k   (q/k projections folded)
  attn = softmax_k(scores)
  out[c,(h,w)] = sum_k attn_k sum_m wpos[m,k] v[2c+m, p'_k],  v = w_v x

Kernel pipeline per core:
  y = G^T x_s                                   (TensorE, f32r)
  V[p'] = w_v x  in column-partition layout      (TensorE, f32r -> bf16)
  per input row r: ST_r[p', pos] = x_r^T y       (TensorE, transposed dense scores)
  E2T = exp(ST)                                  (ScalarE, PSUM->SBUF bf16)
  A_m = E2T * wpos_mask_m                        (VectorE, banded masking)
  out[pos,c] = sum_{r,m} A_m^T V_m  (+ ones-col -> softmax denominator)  (TensorE)
  scale by 1/denom, PE-transpose to [c,pos], DMA out.
"""

import sys

for _p in ("/opt/pypackages", "/opt/trn_rl_repo"):
    if _p not in sys.path:
        sys.path.insert(0, _p)

from contextlib import ExitStack

import ml_dtypes
import numpy as np

import concourse.bacc as bacc
import concourse.bass as bass
import concourse.mybir as mybir
from concourse.bass_utils import run_bass_kernel_spmd
from concourse.tile import TileContext

F32 = mybir.dt.float32
BF16 = mybir.dt.bfloat16

NCORES = 8
CIN = 64
IMG = 128          # input H = W
PIT = IMG + 4      # padded row pitch (pad 2 each side)
OC = 128           # out channels
VCH = 258          # V row pitch: 256 v-channels + 2 ones columns
HO = 64            # output H = W
NPAIR = 32         # output row pairs

# (t, q) -> d  (d = r - 4j for input row r = 4j + d serving pair j)
D_OF = {(0, 0): 4, (0, 1): 0, (1, 0): 1, (2, 0): 2, (2, 1): -2, (3, 0): 3, (3, 1): -1}
# d -> (t, q)
TQ_OF = {d: tq for tq, d in D_OF.items()}


def make_wpos(row_emb, col_emb, mix_emb):
    a = mix_emb.T.astype(np.float64) @ row_emb.astype(np.float64)  # [2,5]
    b = mix_emb.T.astype(np.float64) @ col_emb.astype(np.float64)  # [2,5]
    wp = a[:, :, None] + b[:, None, :]                             # [2,5,5]
    wp = wp - wp.max(axis=0, keepdims=True)
    e = np.exp(wp)
    wp = e / e.sum(axis=0, keepdims=True)
    return wp.reshape(2, 25).astype(np.float32)                    # [m, dh*5+dw]


def make_masks(wpos):
    """wpos-weighted band masks in the transposed (ST) layout.

    Returns [128 (p'=image col), 2 (m), 1024 (t*256 + q*128 + rho*64 + w)] f32."""
    wm = np.zeros((128, 2, 4, 2, 2, 64), np.float32)
    for (t, q), d in D_OF.items():
        for rho in (0, 1):
            dh = d + 2 - 2 * rho
            if not 0 <= dh < 5:
                continue
            for w in range(64):
                for dw in range(5):
                    cimg = 2 * w + dw - 2
                    if 0 <= cimg < 128:
                        wm[cimg, :, t, q, rho, w] = wpos[:, dh * 5 + dw]
    return wm.reshape(128, 2, 1024)


def make_oob():
    """#window entries with out-of-image column, per position in a pair: exp(0)=1 each."""
    oob = np.zeros((128, 1), np.float32)
    for rho in (0, 1):
        for w in range(64):
            cnt = sum(1 for dw in range(5) if not 0 <= 2 * w + dw - 2 < 128)
            oob[rho * 64 + w, 0] = 5.0 * cnt
    return oob


def _ap(t, off, dims, p0=0, pn=None):
    a = t[:]
    np_ = pn if pn is not None else a.ap[0][1]
    return bass.AP(tensor=a.tensor, offset=off + p0 * a.ap[0][0],
                   ap=[[a.ap[0][0], np_]] + [list(d) for d in dims])


def _aph(t, off, dims, p0, pn):
    a = t[p0:p0 + pn]
    return bass.AP(tensor=a.tensor, offset=a.offset + off, ap=[list(a.ap[0])] + [list(d) for d in dims])


def build_nc():
    nc = bacc.Bacc("TRN2", target_bir_lowering=False, debug=False, num_devices=NCORES)

    x_d = nc.dram_tensor("x", [CIN, IMG, IMG], F32, kind="ExternalInput")
    g_d = nc.dram_tensor("g", [128, CIN], BF16, kind="ExternalInput")
    wvt_d = nc.dram_tensor("wvt", [128, 256], BF16, kind="ExternalInput")
    wm_d = nc.dram_tensor("wmask", [128, 2048], BF16, kind="ExternalInput")
    oob_d = nc.dram_tensor("oob", [128, 1], F32, kind="ExternalInput")
    out_d = nc.dram_tensor("out", [HO * HO, OC], F32, kind="ExternalOutput")

    EXP = mybir.ActivationFunctionType.Exp

    with TileContext(nc) as tc, ExitStack() as ctx:
        sg = ctx.enter_context(tc.tile_pool(name="singles", bufs=1))
        x_pad = sg.tile([128, PIT * PIT], BF16)      # padded image, f32
        v_sb = sg.tile([128, PIT * VCH], BF16)       # V + ones cols, padded rows
        y_sb = sg.tile([128, 4096], BF16)            # queries at strided positions
        wm_sb = sg.tile([128, 2048], BF16)
        oob_sb = sg.tile([128, 1], F32)
        g_sb = sg.tile([128, CIN], BF16)
        wvt_sb = sg.tile([128, 256], BF16)

        # constant loads
        nc.sync.dma_start(out=g_sb[:], in_=g_d.ap())
        nc.sync.dma_start(out=wvt_sb[:], in_=wvt_d.ap())
        nc.sync.dma_start(out=wm_sb[:], in_=wm_d.ap())
        nc.sync.dma_start(out=oob_sb[:], in_=oob_d.ap())

        # x_pad borders: rows 0,1 and 130,131; cols 0,1 and 130,131 of interior rows
        nc.vector.memset(_ap(x_pad, 0, [[1, 2 * PIT]]), 0.0)
        nc.vector.memset(_ap(x_pad, 130 * PIT, [[1, 2 * PIT]]), 0.0)
        nc.vector.memset(_ap(x_pad, 2 * PIT, [[PIT, 128], [1, 2]]), 0.0)
        nc.vector.memset(_ap(x_pad, 2 * PIT + 130, [[PIT, 128], [1, 2]]), 0.0)

        # V pad rows (zero) + ones columns
        nc.vector.memset(_ap(v_sb, 0, [[1, 2 * VCH]]), 0.0)
        nc.vector.memset(_ap(v_sb, 130 * VCH, [[1, 2 * VCH]]), 0.0)
        nc.vector.memset(_ap(v_sb, 256, [[VCH, PIT], [1, 2]]), 1.0)

        # x load: 4 chunks of 32 rows, duplicated on both partition halves
        for c8 in range(8):
            nc.gpsimd.dma_start(
                out=_aph(x_pad, (2 + 16 * c8) * PIT + 2, [[PIT, 16], [1, IMG]], 0, 64),
                in_=x_d.ap()[:, 16 * c8:16 * c8 + 16, :],
            )

        def xrow(r, h):
            return x_pad[h:h + 64, (r + 2) * PIT + 2:(r + 2) * PIT + 2 + IMG]

        # ---- phase y + V (shared PSUM pool) ----
        with tc.tile_pool(name="vps", bufs=3, space="PSUM") as vps:
            for ch in range(4):
                yp = vps.tile([128, 1024], F32, tag="vph")
                for i in range(2):
                    hs0 = ch * 16 + i * 8
                    rhs = _aph(x_pad, (2 * hs0 + 2) * PIT + 2, [[2 * PIT, 8], [2, 64]], 0, 64)
                    nc.tensor.matmul(yp[0:CIN, i * 512:(i + 1) * 512], g_sb[0:64, :], rhs,
                                     start=True, stop=True)
                nc.vector.tensor_copy(y_sb[0:CIN, ch * 1024:(ch + 1) * 1024], yp[0:CIN, :])


            for vs in range(32):
                vp = vps.tile([128, 1024], F32, tag="vph")
                for i in range(4):
                    r = vs * 4 + i
                    nc.tensor.matmul(vp[:, i * 256:(i + 1) * 256], xrow(r, 0),
                                     wvt_sb[0:64, :], start=True, stop=True)
                dst = _ap(v_sb, (vs * 4 + 2) * VCH, [[VCH, 4], [1, 256]])
                src = vp[:].rearrange("p (r c) -> p r c", c=256)
                if vs % 3 == 0:
                    nc.vector.tensor_copy(dst, src)
                else:
                    nc.scalar.copy(dst, src)

        # ---- phase pairs: transposed scores slabs + apply ----
        with tc.tile_pool(name="stps", bufs=2, space="PSUM") as stps, \
             tc.tile_pool(name="aps", bufs=3, space="PSUM") as aps, \
             tc.tile_pool(name="e2t", bufs=4) as e2t, \
             tc.tile_pool(name="a0p", bufs=5) as a0p, \
             tc.tile_pool(name="a1p", bufs=5) as a1p, \
             tc.tile_pool(name="outsb", bufs=4) as outsb, \
             tc.tile_pool(name="dens", bufs=6) as dens:

            A = {}

            def make_slab(s):
                stp = stps.tile([128, 1024], F32)
                for t in range(4):
                    r = 4 * s + t
                    if t == 0:
                        jmin, col0 = s - 1, 0
                        if s == 0:
                            jmin, col0 = 0, 128
                    else:
                        jmin, col0 = s, 0
                    n = min(256 - col0, (NPAIR - jmin) * 128)
                    h = 0
                    rhs = y_sb[h:h + 64, jmin * 128: jmin * 128 + n]
                    nc.tensor.matmul(stp[:, t * 256 + col0: t * 256 + col0 + n],
                                     xrow(r, h), rhs, start=True, stop=True)
                e2 = e2t.tile([128, 1024], BF16)
                nc.scalar.activation(out=e2[:], in_=stp[:], func=EXP)
                a0 = a0p.tile([128, 1024], BF16)
                a1 = a1p.tile([128, 1024], BF16)
                nc.vector.tensor_mul(a0[:], e2[:], wm_sb[:, 0:1024])
                nc.vector.tensor_mul(a1[:], e2[:], wm_sb[:, 1024:2048])
                A[s] = (a0, a1)
                A.pop(s - 3, None)

            def apply_pair(j):
                ap_ps = aps.tile([128, 130], F32)
                ops = [(d, m) for d in (0, 1, 2, -2, -1, 3, 4) for m in (0, 1)]
                for idx, (d, m) in enumerate(ops):
                    r = 4 * j + d
                    if r < 0 or r >= IMG:
                        t, q = TQ_OF[d]
                        off = m * 1024 + t * 256 + q * 128
                        src = wm_sb
                    else:
                        sl, t = r // 4, r % 4
                        if t == 0:
                            q = 0 if j == sl - 1 else 1
                        elif t == 1:
                            q = 0
                        else:
                            q = 0 if j == sl else 1
                        off = t * 256 + q * 128
                        src = A[sl][m]
                    lhsT = src[:, off: off + 128]
                    out_ps = ap_ps[:, 0:129]
                    rhs = _ap(v_sb, (r + 2) * VCH + m, [[2, 129]])
                    nc.tensor.matmul(out_ps, lhsT, rhs,
                                     start=(idx == 0), stop=(idx == len(ops) - 1),
                                     skip_group_check=True)
                den = dens.tile([128, 1], F32)
                nc.vector.tensor_add(den[:], ap_ps[:, 128:129], oob_sb[:])
                rec = dens.tile([128, 1], F32)
                nc.vector.reciprocal(rec[:], den[:])
                o_sb = outsb.tile([128, 128], F32)
                nc.scalar.activation(out=o_sb[:], in_=ap_ps[:, 0:128],
                                     func=mybir.ActivationFunctionType.Copy, scale=rec[:])
                nc.sync.dma_start(out=out_d.ap()[j * 128:(j + 1) * 128, :], in_=o_sb[:])

            for s in range(NPAIR):
                make_slab(s)
                if s >= 1:
                    apply_pair(s - 1)
            apply_pair(NPAIR - 1)


    nc.compile()
    return nc


_NC_CACHE = None


def kernel(x, w_q, w_k, w_v, row_emb, col_emb, mix_emb):
    global _NC_CACHE
    x = np.asarray(x, np.float32)
    w_q = np.asarray(w_q, np.float32)
    w_k = np.asarray(w_k, np.float32)
    w_v = np.asarray(w_v, np.float32)
    row_emb = np.asarray(row_emb, np.float32)
    col_emb = np.asarray(col_emb, np.float32)
    mix_emb = np.asarray(mix_emb, np.float32)

    G = (w_q.T @ w_k).astype(np.float32)
    wvt = np.ascontiguousarray(w_v.T).astype(np.float32)      # [64, 256]
    wpos = make_wpos(row_emb, col_emb, mix_emb)
    wmask = make_masks(wpos).reshape(128, 2048).astype(ml_dtypes.bfloat16)
    oob = make_oob()
    ident = np.eye(128, dtype=np.float32)

    if _NC_CACHE is None:
        _NC_CACHE = build_nc()
    nc = _NC_CACHE

    in_maps = []
    for b in range(NCORES):
        in_maps.append({
            "x": np.ascontiguousarray(x[b]),
            "g": G,
            "wvt": wvt,
            "wmask": wmask,
            "oob": oob,
        })
    res = run_bass_kernel_spmd(nc, in_maps, core_ids=list(range(NCORES)))
    out = np.stack([res.results[b]["out"].T.reshape(OC, HO, HO) for b in range(NCORES)])
    return out.astype(np.float32)


# revision 20
# speedup vs baseline: 1.6799x; 1.0639x over previous
"""Trainium2 Bass kernel for nn_AttentionStem (5x5 local attention stem, stride 2).

Self-contained: hardcodes shapes B=8, CIN=64, H=W=128, OUT_CH=128, M=2, K=5.
Data-parallel over batch: one batch element per NeuronCore (8 cores).

Math (per batch):
  scores[k,(h,w)] = x_s(2h,2w)^T G x(p'_k),  G = w_q^T w_k   (q/k projections folded)
  attn = softmax_k(scores)
  out[c,(h,w)] = sum_k attn_k sum_m wpos[m,k] v[2c+m, p'_k],  v = w_v x

Kernel pipeline per core:
  y = G^T x_s                                   (TensorE, f32r)
  V[p'] = w_v x  in column-partition layout      (TensorE, f32r -> bf16)
  per input row r: ST_r[p', pos] = x_r^T y       (TensorE, transposed dense scores)
  E2T = exp(ST)                                  (ScalarE, PSUM->SBUF bf16)
  A_m = E2T * wpos_mask_m                        (VectorE, banded masking)
  out[pos,c] = sum_{r,m} A_m^T V_m  (+ ones-col -> softmax denominator)  (TensorE)
  scale by 1/denom, PE-transpose to [c,pos], DMA out.
"""

import sys

for _p in ("/opt/pypackages", "/opt/trn_rl_repo"):
    if _p not in sys.path:
        sys.path.insert(0, _p)

from contextlib import ExitStack

import ml_dtypes
import numpy as np

import concourse.bacc as bacc
import concourse.bass as bass
import concourse.mybir as mybir
from concourse.bass_utils import run_bass_kernel_spmd
from concourse.tile import TileContext

F32 = mybir.dt.float32
BF16 = mybir.dt.bfloat16

NCORES = 8
CIN = 64
IMG = 128          # input H = W
PIT = IMG + 4      # padded row pitch (pad 2 each side)
OC = 128           # out channels
VCH = 258          # V row pitch: 256 v-channels + 2 ones columns
HO = 64            # output H = W
NPAIR = 32         # output row pairs

# (t, q) -> d  (d = r - 4j for input row r = 4j + d serving pair j)
D_OF = {(0, 0): 4, (0, 1): 0, (1, 0): 1, (2, 0): 2, (2, 1): -2, (3, 0): 3, (3, 1): -1}
# d -> (t, q)
TQ_OF = {d: tq for tq, d in D_OF.items()}


def make_wpos(row_emb, col_emb, mix_emb):
    a = mix_emb.T.astype(np.float64) @ row_emb.astype(np.float64)  # [2,5]
    b = mix_emb.T.astype(np.float64) @ col_emb.astype(np.float64)  # [2,5]
    wp = a[:, :, None] + b[:, None, :]                             # [2,5,5]
    wp = wp - wp.max(axis=0, keepdims=True)
    e = np.exp(wp)
    wp = e / e.sum(axis=0, keepdims=True)
    return wp.reshape(2, 25).astype(np.float32)                    # [m, dh*5+dw]


def make_masks(wpos):
    """wpos-weighted band masks in the transposed (ST) layout.

    Returns [128 (p'=image col), 2 (m), 1024 (t*256 + q*128 + rho*64 + w)] f32."""
    wm = np.zeros((128, 2, 4, 2, 2, 64), np.float32)
    for (t, q), d in D_OF.items():
        for rho in (0, 1):
            dh = d + 2 - 2 * rho
            if not 0 <= dh < 5:
                continue
            for w in range(64):
                for dw in range(5):
                    cimg = 2 * w + dw - 2
                    if 0 <= cimg < 128:
                        wm[cimg, :, t, q, rho, w] = wpos[:, dh * 5 + dw]
    return wm.reshape(128, 2, 1024)


def make_oob():
    """#window entries with out-of-image column, per position in a pair: exp(0)=1 each."""
    oob = np.zeros((128, 1), np.float32)
    for rho in (0, 1):
        for w in range(64):
            cnt = sum(1 for dw in range(5) if not 0 <= 2 * w + dw - 2 < 128)
            oob[rho * 64 + w, 0] = 5.0 * cnt
    return oob


def _ap(t, off, dims, p0=0, pn=None):
    a = t[:]
    np_ = pn if pn is not None else a.ap[0][1]
    return bass.AP(tensor=a.tensor, offset=off + p0 * a.ap[0][0],
                   ap=[[a.ap[0][0], np_]] + [list(d) for d in dims])


def _aph(t, off, dims, p0, pn):
    a = t[p0:p0 + pn]
    return bass.AP(tensor=a.tensor, offset=a.offset + off, ap=[list(a.ap[0])] + [list(d) for d in dims])


def build_nc():
    nc = bacc.Bacc("TRN2", target_bir_lowering=False, debug=False, num_devices=NCORES)

    x_d = nc.dram_tensor("x", [CIN, IMG, IMG], F32, kind="ExternalInput")
    g_d = nc.dram_tensor("g", [128, CIN], BF16, kind="ExternalInput")
    wvt_d = nc.dram_tensor("wvt", [128, 256], BF16, kind="ExternalInput")
    wm_d = nc.dram_tensor("wmask", [128, 2048], BF16, kind="ExternalInput")
    oob_d = nc.dram_tensor("oob", [128, 1], F32, kind="ExternalInput")
    out_d = nc.dram_tensor("out", [HO * HO, OC], F32, kind="ExternalOutput")

    EXP = mybir.ActivationFunctionType.Exp

    with TileContext(nc) as tc, ExitStack() as ctx:
        sg = ctx.enter_context(tc.tile_pool(name="singles", bufs=1))
        x_pad = sg.tile([128, PIT * PIT], BF16)      # padded image, f32
        v_sb = sg.tile([128, PIT * VCH], BF16)       # V + ones cols, padded rows
        y_sb = sg.tile([128, 4096], BF16)            # queries at strided positions
        wm_sb = sg.tile([128, 2048], BF16)
        oob_sb = sg.tile([128, 1], F32)
        g_sb = sg.tile([128, CIN], BF16)
        wvt_sb = sg.tile([128, 256], BF16)

        # constant loads
        nc.sync.dma_start(out=g_sb[:], in_=g_d.ap())
        nc.sync.dma_start(out=wvt_sb[:], in_=wvt_d.ap())
        nc.sync.dma_start(out=wm_sb[:], in_=wm_d.ap())
        nc.sync.dma_start(out=oob_sb[:], in_=oob_d.ap())

        # x_pad borders: rows 0,1 and 130,131; cols 0,1 and 130,131 of interior rows
        nc.vector.memset(_ap(x_pad, 0, [[1, 2 * PIT]]), 0.0)
        nc.vector.memset(_ap(x_pad, 130 * PIT, [[1, 2 * PIT]]), 0.0)
        nc.vector.memset(_ap(x_pad, 2 * PIT, [[PIT, 128], [1, 2]]), 0.0)
        nc.vector.memset(_ap(x_pad, 2 * PIT + 130, [[PIT, 128], [1, 2]]), 0.0)

        # V pad rows (zero) + ones columns
        nc.vector.memset(_ap(v_sb, 0, [[1, 2 * VCH]]), 0.0)
        nc.vector.memset(_ap(v_sb, 130 * VCH, [[1, 2 * VCH]]), 0.0)
        nc.vector.memset(_ap(v_sb, 256, [[VCH, PIT], [1, 2]]), 1.0)

        # x load: 4 chunks of 32 rows, duplicated on both partition halves
        for c8 in range(8):
            nc.gpsimd.dma_start(
                out=_aph(x_pad, (2 + 16 * c8) * PIT + 2, [[PIT, 16], [1, IMG]], 0, 64),
                in_=x_d.ap()[:, 16 * c8:16 * c8 + 16, :],
            )

        def xrow(r, h):
            return x_pad[h:h + 64, (r + 2) * PIT + 2:(r + 2) * PIT + 2 + IMG]

        # ---- phase y + V (shared PSUM pool) ----
        with tc.tile_pool(name="vps", bufs=3, space="PSUM") as vps:
            for vs in range(32):
                vp = vps.tile([128, 1024], F32, tag="vph")
                for i in range(4):
                    r = vs * 4 + i
                    nc.tensor.matmul(vp[:, i * 256:(i + 1) * 256], xrow(r, 0),
                                     wvt_sb[0:64, :], start=True, stop=True)
                dst = _ap(v_sb, (vs * 4 + 2) * VCH, [[VCH, 4], [1, 256]])
                src = vp[:].rearrange("p (r c) -> p r c", c=256)
                if vs % 3 == 0:
                    nc.vector.tensor_copy(dst, src)
                else:
                    nc.scalar.copy(dst, src)

            for ch in range(4):
                yp = vps.tile([128, 1024], F32, tag="vph")
                for i in range(2):
                    hs0 = ch * 16 + i * 8
                    rhs = _aph(x_pad, (2 * hs0 + 2) * PIT + 2, [[2 * PIT, 8], [2, 64]], 0, 64)
                    nc.tensor.matmul(yp[0:CIN, i * 512:(i + 1) * 512], g_sb[0:64, :], rhs,
                                     start=True, stop=True)
                nc.vector.tensor_copy(y_sb[0:CIN, ch * 1024:(ch + 1) * 1024], yp[0:CIN, :])

        # ---- phase pairs: transposed scores slabs + apply ----
        with tc.tile_pool(name="stps", bufs=2, space="PSUM") as stps, \
             tc.tile_pool(name="aps", bufs=3, space="PSUM") as aps, \
             tc.tile_pool(name="e2t", bufs=4) as e2t, \
             tc.tile_pool(name="a0p", bufs=5) as a0p, \
             tc.tile_pool(name="a1p", bufs=5) as a1p, \
             tc.tile_pool(name="outsb", bufs=4) as outsb, \
             tc.tile_pool(name="dens", bufs=6) as dens:

            A = {}

            def make_slab(s):
                stp = stps.tile([128, 1024], F32)
                for t in range(4):
                    r = 4 * s + t
                    if t == 0:
                        jmin, col0 = s - 1, 0
                        if s == 0:
                            jmin, col0 = 0, 128
                    else:
                        jmin, col0 = s, 0
                    n = min(256 - col0, (NPAIR - jmin) * 128)
                    h = 0
                    rhs = y_sb[h:h + 64, jmin * 128: jmin * 128 + n]
                    nc.tensor.matmul(stp[:, t * 256 + col0: t * 256 + col0 + n],
                                     xrow(r, h), rhs, start=True, stop=True)
                e2 = e2t.tile([128, 1024], BF16)
                nc.scalar.activation(out=e2[:], in_=stp[:], func=EXP)
                a0 = a0p.tile([128, 1024], BF16)
                a1 = a1p.tile([128, 1024], BF16)
                nc.vector.tensor_mul(a0[:], e2[:], wm_sb[:, 0:1024])
                nc.vector.tensor_mul(a1[:], e2[:], wm_sb[:, 1024:2048])
                A[s] = (a0, a1)
                A.pop(s - 3, None)

            def apply_pair(j):
                ap_ps = aps.tile([128, 130], F32)
                ops = [(d, m) for d in (0, 1, 2, -2, -1, 3, 4) for m in (0, 1)]
                for idx, (d, m) in enumerate(ops):
                    r = 4 * j + d
                    if r < 0 or r >= IMG:
                        t, q = TQ_OF[d]
                        off = m * 1024 + t * 256 + q * 128
                        src = wm_sb
                    else:
                        sl, t = r // 4, r % 4
                        if t == 0:
                            q = 0 if j == sl - 1 else 1
                        elif t == 1:
                            q = 0
                        else:
                            q = 0 if j == sl else 1
                        off = t * 256 + q * 128
                        src = A[sl][m]
                    lhsT = src[:, off: off + 128]
                    out_ps = ap_ps[:, 0:129]
                    rhs = _ap(v_sb, (r + 2) * VCH + m, [[2, 129]])
                    nc.tensor.matmul(out_ps, lhsT, rhs,
                                     start=(idx == 0), stop=(idx == len(ops) - 1),
                                     skip_group_check=True)
                den = dens.tile([128, 1], F32)
                nc.vector.tensor_add(den[:], ap_ps[:, 128:129], oob_sb[:])
                rec = dens.tile([128, 1], F32)
                nc.vector.reciprocal(rec[:], den[:])
                o_sb = outsb.tile([128, 128], F32)
                nc.scalar.activation(out=o_sb[:], in_=ap_ps[:, 0:128],
                                     func=mybir.ActivationFunctionType.Copy, scale=rec[:])
                nc.sync.dma_start(out=out_d.ap()[j * 128:(j + 1) * 128, :], in_=o_sb[:])

            for s in range(NPAIR):
                make_slab(s)
                if s >= 1:
                    apply_pair(s - 1)
            apply_pair(NPAIR - 1)


    nc.compile()
    return nc


_NC_CACHE = None


def kernel(x, w_q, w_k, w_v, row_emb, col_emb, mix_emb):
    global _NC_CACHE
    x = np.asarray(x, np.float32)
    w_q = np.asarray(w_q, np.float32)
    w_k = np.asarray(w_k, np.float32)
    w_v = np.asarray(w_v, np.float32)
    row_emb = np.asarray(row_emb, np.float32)
    col_emb = np.asarray(col_emb, np.float32)
    mix_emb = np.asarray(mix_emb, np.float32)

    G = (w_q.T @ w_k).astype(np.float32)
    wvt = np.ascontiguousarray(w_v.T).astype(np.float32)      # [64, 256]
    wpos = make_wpos(row_emb, col_emb, mix_emb)
    wmask = make_masks(wpos).reshape(128, 2048).astype(ml_dtypes.bfloat16)
    oob = make_oob()
    ident = np.eye(128, dtype=np.float32)

    if _NC_CACHE is None:
        _NC_CACHE = build_nc()
    nc = _NC_CACHE

    in_maps = []
    for b in range(NCORES):
        in_maps.append({
            "x": np.ascontiguousarray(x[b]),
            "g": G,
            "wvt": wvt,
            "wmask": wmask,
            "oob": oob,
        })
    res = run_bass_kernel_spmd(nc, in_maps, core_ids=list(range(NCORES)))
    out = np.stack([res.results[b]["out"].T.reshape(OC, HO, HO) for b in range(NCORES)])
    return out.astype(np.float32)
